# revision 1
# baseline (speedup 1.0000x reference)
"""DeepfakeGNN (2x GCNConv + mean-pool + fc) on 8 Trainium2 NeuronCores.

Sharding: graphs are split 16-per-core (batch is sorted, so each core owns a
contiguous node range). Edges are assigned to the core owning their dst node.
Per layer: each core computes X@W for its own nodes, AllGathers the result,
then aggregates messages for its own dst nodes via dma_gather (rows by src
index) + one-hot segment matmuls on the tensor engine. Pool + fc are local;
the host concatenates the per-core [16] outputs.

Self-contained: only numpy + concourse (preinstalled on the PYTHONPATH).
"""
import numpy as np

import concourse.mybir as mybir
from concourse import bacc
from concourse.bass_utils import run_bass_kernel_spmd
from concourse.masks import make_identity
from concourse.tile import TileContext

NC = 8          # cores
N = 20000       # nodes
D_IN = 512
DH = 256
G = 128         # graphs
GP = G // NC    # graphs per core

FP32 = mybir.dt.float32
BF16 = mybir.dt.bfloat16
I16 = mybir.dt.int16
I32 = mybir.dt.int32


# ---------------------------------------------------------------- host prep

def _wrap16(arr, cols):
    """Lay out a flat int array [cols*16] -> [128, cols] in dma_gather idx
    order (idx j at [j%16, j//16], replicated across the 8 q7 cores)."""
    a = arr.reshape(cols, 16).T  # [16, cols]
    return np.ascontiguousarray(np.tile(a, (8, 1)))


def prep(x, edge_index, batch, W1, b1, W2, b2, w_fc, b_fc):
    x = np.asarray(x, dtype=np.float32)
    ei = np.asarray(edge_index).astype(np.int64)
    batch = np.asarray(batch).astype(np.int64)
    W1 = np.asarray(W1, dtype=np.float32)
    b1 = np.asarray(b1, dtype=np.float32)
    W2 = np.asarray(W2, dtype=np.float32)
    b2 = np.asarray(b2, dtype=np.float32)
    w_fc = np.asarray(w_fc, dtype=np.float32)
    b_fc = np.asarray(b_fc, dtype=np.float32)

    n = x.shape[0]
    loops = np.arange(n, dtype=np.int64)
    src = np.concatenate([ei[0], loops])
    dst = np.concatenate([ei[1], loops])

    deg = np.bincount(dst, minlength=n).astype(np.float32)
    dinv = np.where(deg > 0, 1.0 / np.sqrt(deg, dtype=np.float32), 0.0).astype(np.float32)
    coef = (dinv[src] * dinv[dst]).astype(np.float32)

    # node ranges per core (graphs [c*GP, (c+1)*GP))
    bounds = np.searchsorted(batch, np.arange(0, G + 1, GP))
    n_c = bounds[1:] - bounds[:-1]
    n_pad = int(int(np.ceil(n_c.max() / 128.0)) * 128)
    T = n_pad // 128

    owner = (batch // GP).astype(np.int64)        # owning core per node
    gidx = owner * n_pad + (np.arange(n) - bounds[owner])  # gathered row index

    # per-core edge streams grouped by dst tile
    per_core = []
    for c in range(NC):
        m = (dst >= bounds[c]) & (dst < bounds[c + 1])
        es, ed, ec = src[m], dst[m], coef[m]
        dl = ed - bounds[c]
        order = np.argsort(dl, kind="stable")
        es, ec, dl = es[order], ec[order], dl[order]
        tb = np.searchsorted(dl, np.arange(0, n_pad + 1, 128))
        per_core.append((es, ec, dl, tb))

    # shared chunk schedule: CH[t] = max over cores of chunks needed for tile t
    CH = []
    for t in range(T):
        mx = 1
        for c in range(NC):
            _, _, _, tb = per_core[c]
            cnt = int(tb[t + 1] - tb[t])
            mx = max(mx, (cnt + 127) // 128)
        CH.append(mx)
    TOT = sum(CH)

    # per-graph 1/count for mean pooling
    gcnt = np.bincount(batch, minlength=G).astype(np.float32)
    ginv = 1.0 / np.maximum(gcnt, 1.0)

    in_maps = []
    for c in range(NC):
        es, ec, dl, tb = per_core[c]
        gsrc = np.zeros(TOT * 128, dtype=np.int16)
        dlv = np.zeros(TOT * 128, dtype=np.float32)
        cfv = np.zeros(TOT * 128, dtype=np.float32)
        off = 0
        for t in range(T):
            a, b = int(tb[t]), int(tb[t + 1])
            cnt = b - a
            gsrc[off:off + cnt] = gidx[es[a:b]]
            dlv[off:off + cnt] = (dl[a:b] - t * 128).astype(np.float32)
            cfv[off:off + cnt] = ec[a:b]
            off += CH[t] * 128
        gidx_sb = _wrap16(gsrc, TOT * 8).astype(np.int16)
        dlcf = np.zeros((128, 2 * TOT), dtype=np.float32)
        dlcf[:, :TOT] = dlv.reshape(TOT, 128).T
        dlcf[:, TOT:] = cfv.reshape(TOT, 128).T

        lo, hi = int(bounds[c]), int(bounds[c + 1])
        xT = np.zeros((D_IN, n_pad), dtype=np.float32)
        xT[:, : hi - lo] = x[lo:hi].T

        pp = np.zeros((128, T * 16), dtype=np.float32)
        gl = batch[lo:hi] - c * GP            # local graph id per node
        giv = ginv[c * GP:(c + 1) * GP]
        rows = np.arange(hi - lo)
        pp[rows % 128, (rows // 128) * 16 + gl] = giv[gl]

        bias = np.zeros((1, 2 * DH), dtype=np.float32)
        bias[0, :DH] = b1
        bias[0, DH:] = b2

        in_maps.append({
            "xT": xT,
            "w1": W1,
            "w2": W2,
            "pp": pp,
            "bias": bias,
            "wfc": np.ascontiguousarray(np.broadcast_to(w_fc[:, 0][None, :], (16, DH)).astype(np.float32)),
            "bfc": np.full((16, 1), float(b_fc[0]), dtype=np.float32),
            "gidx": gidx_sb,
            "dlcf": dlcf,
        })

    return in_maps, n_pad, tuple(CH)


# ---------------------------------------------------------------- device build

_CACHE = {}


def build(n_pad, CH):
    key = (n_pad, CH)
    if key in _CACHE:
        return _CACHE[key]
    T = n_pad // 128
    TOT = sum(CH)

    nc = bacc.Bacc(dynamic_dma_scratch_size=131072)
    xT_in = nc.dram_tensor("xT", [D_IN, n_pad], FP32, kind="ExternalInput")
    w1_in = nc.dram_tensor("w1", [D_IN, DH], FP32, kind="ExternalInput")
    w2_in = nc.dram_tensor("w2", [DH, DH], FP32, kind="ExternalInput")
    pp_in = nc.dram_tensor("pp", [128, T * 16], FP32, kind="ExternalInput")
    bias_in = nc.dram_tensor("bias", [1, 2 * DH], FP32, kind="ExternalInput")
    wfc_in = nc.dram_tensor("wfc", [16, DH], FP32, kind="ExternalInput")
    bfc_in = nc.dram_tensor("bfc", [16, 1], FP32, kind="ExternalInput")
    gidx_in = nc.dram_tensor("gidx", [128, TOT * 8], I16, kind="ExternalInput")
    dlcf_in = nc.dram_tensor("dlcf", [128, 2 * TOT], FP32, kind="ExternalInput")
    out = nc.dram_tensor("out", [16, 1], FP32, kind="ExternalOutput")

    zloc = [nc.dram_tensor(f"z{l}loc", [n_pad, DH], BF16) for l in (1, 2)]
    zfull = [nc.dram_tensor(f"z{l}full", [NC * n_pad, DH], BF16, addr_space="Shared")
             for l in (1, 2)]

    with TileContext(nc) as tc:
        with (
            tc.tile_pool(name="const", bufs=1) as const,
            tc.tile_pool(name="zp", bufs=3) as zp,
            tc.tile_pool(name="gp", bufs=4) as gp,
            tc.tile_pool(name="sp", bufs=6) as sp,
            tc.tile_pool(name="hp", bufs=3) as hp,
            tc.tile_pool(name="tp", bufs=4) as tp,
            tc.tile_pool(name="fp", bufs=1) as fp,
            tc.tile_pool(name="psA", bufs=2, space="PSUM") as psA,
            tc.tile_pool(name="psM", bufs=3, space="PSUM") as psM,
            tc.tile_pool(name="psT", bufs=2, space="PSUM") as psT,
            tc.tile_pool(name="psP", bufs=1, space="PSUM") as psP,
        ):
            # ---- constant loads
            xT_sb = const.tile([128, 4, n_pad], FP32)
            for k in range(4):
                nc.sync.dma_start(out=xT_sb[:, k, :], in_=xT_in[k * 128:(k + 1) * 128, :])
            w1_sb = const.tile([128, 4, DH], FP32)
            for k in range(4):
                nc.sync.dma_start(out=w1_sb[:, k, :], in_=w1_in[k * 128:(k + 1) * 128, :])
            w2_sb = const.tile([128, 2, DH], FP32)
            for k in range(2):
                nc.sync.dma_start(out=w2_sb[:, k, :], in_=w2_in[k * 128:(k + 1) * 128, :])
            pp_sb = const.tile([128, T * 16], FP32)
            nc.sync.dma_start(out=pp_sb[:], in_=pp_in[:])
            bias_sb = const.tile([1, 2 * DH], FP32)
            nc.sync.dma_start(out=bias_sb[:], in_=bias_in[:])
            wfc_sb = const.tile([16, DH], FP32)
            nc.sync.dma_start(out=wfc_sb[:], in_=wfc_in[:])
            bfc_sb = const.tile([16, 1], FP32)
            nc.sync.dma_start(out=bfc_sb[:], in_=bfc_in[:])
            gidx_sb = const.tile([128, TOT * 8], I16)
            nc.sync.dma_start(out=gidx_sb[:], in_=gidx_in[:])
            dlcf_sb = const.tile([128, 2 * TOT], FP32)
            nc.sync.dma_start(out=dlcf_sb[:], in_=dlcf_in[:])

            ident = const.tile([128, 128], FP32)
            make_identity(nc, ident[:])
            ones_sb = const.tile([1, 128], BF16)
            nc.vector.memset(ones_sb[:], 1.0)
            bias_bf = const.tile([1, 2 * DH], BF16)
            nc.vector.tensor_copy(bias_bf[:], bias_sb[:])
            iota_i = const.tile([128, 128], I32)
            nc.gpsimd.iota(iota_i[:], pattern=[[1, 128]], base=0, channel_multiplier=0)
            iota_f = const.tile([128, 128], BF16)
            nc.vector.tensor_copy(iota_f[:], iota_i[:])

            # ---- phase A: z1 = x @ W1 (node-major tiles)
            for t in range(T):
                acc = psA.tile([128, DH], FP32, space="PSUM", tag="psA")
                for k in range(4):
                    nc.tensor.matmul(
                        out=acc[:], lhsT=xT_sb[:, k, t * 128:(t + 1) * 128],
                        rhs=w1_sb[:, k, :], start=(k == 0), stop=(k == 3))
                z1s = zp.tile([128, DH], BF16, tag="z")
                nc.vector.tensor_copy(z1s[:], acc[:])
                nc.sync.dma_start(out=zloc[0][t * 128:(t + 1) * 128, :], in_=z1s[:])

            # ---- two GCN message-passing layers
            GRP = 8  # chunks per dma_gather (1024 idxs; bounded by SWDGE ring)
            for layer in range(2):
                nc.gpsimd.collective_compute(
                    "AllGather", mybir.AluOpType.bypass,
                    ins=[zloc[layer][:]], outs=[zfull[layer][:]],
                    replica_groups=[list(range(NC))])

                gtile = [None, None]  # (group id, tile)

                def get_msg(q, layer=layer, gtile=gtile):
                    grp = q // GRP
                    if gtile[0] != grp:
                        sz = min(GRP, TOT - grp * GRP)
                        gt = gp.tile([128, sz, DH], BF16, tag="g")
                        nc.gpsimd.dma_gather(
                            out_ap=gt[:],
                            in_ap=zfull[layer][:],
                            idxs_ap=gidx_sb[:, grp * GRP * 8:(grp * GRP + sz) * 8],
                            num_idxs=sz * 128,
                            num_idxs_reg=sz * 128,
                            elem_size=DH,
                        )
                        gtile[0], gtile[1] = grp, gt
                    return gtile[1][:, q % GRP, :]

                off = 0
                for t in range(T):
                    ch = CH[t]
                    agg = psM.tile([128, DH], FP32, space="PSUM", tag="psM")
                    for j in range(ch):
                        msg = get_msg(off + j)
                        S = sp.tile([128, 128], BF16, tag="S")
                        nc.vector.tensor_scalar(
                            out=S[:], in0=iota_f[:],
                            scalar1=dlcf_sb[:, off + j:off + j + 1],
                            scalar2=dlcf_sb[:, TOT + off + j:TOT + off + j + 1],
                            op0=mybir.AluOpType.is_equal,
                            op1=mybir.AluOpType.mult)
                        nc.tensor.matmul(out=agg[:], lhsT=S[:], rhs=msg,
                                         start=(j == 0), stop=False)
                    # += bias (broadcast over dst rows), then close the group
                    nc.tensor.matmul(
                        out=agg[:], lhsT=ones_sb[:],
                        rhs=bias_bf[0:1, layer * DH:(layer + 1) * DH],
                        start=False, stop=True)
                    h = hp.tile([128, DH], FP32, tag="h")
                    nc.vector.tensor_scalar_max(h[:], agg[:], 0.0)  # relu

                    if layer == 0:
                        # z2 tile = relu(h) @ W2 via PE transpose of h
                        hT = []
                        for half in range(2):
                            pt = psT.tile([128, 128], FP32, space="PSUM", tag="psT")
                            nc.tensor.transpose(
                                out=pt[:], in_=h[:, half * 128:(half + 1) * 128],
                                identity=ident[:])
                            ht = tp.tile([128, 128], FP32, tag="hT")
                            nc.vector.tensor_copy(ht[:], pt[:])
                            hT.append(ht)
                        accz = psA.tile([128, DH], FP32, space="PSUM", tag="psA")
                        for half in range(2):
                            nc.tensor.matmul(out=accz[:], lhsT=hT[half][:],
                                             rhs=w2_sb[:, half, :],
                                             start=(half == 0), stop=(half == 1))
                        z2s = zp.tile([128, DH], BF16, tag="z")
                        nc.vector.tensor_copy(z2s[:], accz[:])
                        nc.sync.dma_start(out=zloc[1][t * 128:(t + 1) * 128, :], in_=z2s[:])
                    else:
                        # mean-pool: pooled[16, DH] += pp_t.T @ h
                        if t == 0:
                            pool_acc = psP.tile([16, DH], FP32, space="PSUM", tag="psP")
                        nc.tensor.matmul(out=pool_acc[:],
                                         lhsT=pp_sb[:, t * 16:(t + 1) * 16],
                                         rhs=h[:], start=(t == 0), stop=(t == T - 1),
                                         skip_group_check=True)
                    off += ch

            # ---- fc head: out = pooled @ w_fc + b_fc
            pooled = fp.tile([16, DH], FP32)
            nc.vector.tensor_copy(pooled[:], pool_acc[:])
            prod = fp.tile([16, DH], FP32)
            nc.vector.tensor_tensor(out=prod[:], in0=pooled[:], in1=wfc_sb[:],
                                    op=mybir.AluOpType.mult)
            red = fp.tile([16, 1], FP32)
            nc.vector.reduce_sum(red[:], prod[:], axis=mybir.AxisListType.X)
            outv = fp.tile([16, 1], FP32)
            nc.vector.tensor_scalar_add(outv[:], red[:], bfc_sb[:])
            nc.sync.dma_start(out=out[:], in_=outv[:])

    nc.finalize()
    _CACHE[key] = nc
    return nc


# ---------------------------------------------------------------- entry points

def _run(inputs, trace=False):
    in_maps, n_pad, CH = prep(**inputs)
    nc = build(n_pad, CH)
    r = run_bass_kernel_spmd(nc, in_maps, list(range(NC)), trace=trace)
    parts = [r.results[c]["out"][:, 0] for c in range(NC)]
    return np.concatenate(parts).astype(np.float32), r


def kernel(**inputs):
    out, _ = _run(inputs, trace=False)
    return out


def kernel_traced(**inputs):
    out, r = _run(inputs, trace=True)
    return out, r



# revision 19
# speedup vs baseline: 1.5717x; 1.5717x over previous
"""DeepfakeGNN (2x GCNConv + mean-pool + fc) on 8 Trainium2 NeuronCores.

Scatter/ReduceScatter dataflow: graphs are split 16-per-core (batch is sorted,
so each core owns a contiguous node range). Edges are assigned to the core
owning their SRC node. Per layer: each core computes z = H @ W for its own
nodes (bf16), scatters edge messages into a full-size [8*n_pad, 256] partial
accumulator via dma_gather (rows by local src index) + one-hot segment matmuls
on the tensor engine, then a single ReduceScatter(add) returns each core the
summed aggregation for its own rows. Self-loops and bias are applied post-RS
as a per-row scaled copy of local z plus a broadcast bias (no scatter slots).
Pool + fc are local; the host concatenates the per-core [16] outputs.

Self-contained: only numpy + ml_dtypes + concourse (preinstalled).
"""
import numpy as np
import ml_dtypes

import concourse.mybir as mybir
from concourse import bacc
from concourse.bass_utils import run_bass_kernel_spmd
from concourse.masks import make_identity
from concourse.tile import TileContext

NC = 8          # cores
N = 20000       # nodes
D_IN = 512
DH = 256
G = 128         # graphs
GP = G // NC    # graphs per core

FP32 = mybir.dt.float32
BF16 = mybir.dt.bfloat16
I16 = mybir.dt.int16
I32 = mybir.dt.int32

BF = ml_dtypes.bfloat16


# ---------------------------------------------------------------- host prep

def _wrap16(arr, cols):
    """Lay out a flat int array [cols*16] -> [128, cols] in dma_gather idx
    order (idx j at [j%16, j//16], replicated across the 8 q7 cores)."""
    a = arr.reshape(cols, 16).T  # [16, cols]
    return np.ascontiguousarray(np.tile(a, (8, 1)))


def prep(x, edge_index, batch, W1, b1, W2, b2, w_fc, b_fc):
    x = np.asarray(x, dtype=np.float32)
    ei = np.asarray(edge_index).astype(np.int64)
    batch = np.asarray(batch).astype(np.int64)
    W1 = np.asarray(W1, dtype=np.float32)
    b1 = np.asarray(b1, dtype=np.float32)
    W2 = np.asarray(W2, dtype=np.float32)
    b2 = np.asarray(b2, dtype=np.float32)
    w_fc = np.asarray(w_fc, dtype=np.float32)
    b_fc = np.asarray(b_fc, dtype=np.float32)

    n = x.shape[0]
    src, dst = ei[0], ei[1]

    # degree includes self-loop (reference concatenates loops)
    deg = (np.bincount(dst, minlength=n) + 1.0).astype(np.float32)
    dinv = (1.0 / np.sqrt(deg, dtype=np.float32)).astype(np.float32)
    coef = (dinv[src] * dinv[dst]).astype(np.float32)
    selfc = (dinv * dinv).astype(np.float32)

    # node ranges per core (graphs [c*GP, (c+1)*GP))
    bounds = np.searchsorted(batch, np.arange(0, G + 1, GP))
    n_c = bounds[1:] - bounds[:-1]
    n_pad = int(int(np.ceil(n_c.max() / 128.0)) * 128)
    T = n_pad // 128
    TF = NC * T

    owner = (batch // GP).astype(np.int64)        # owning core per node
    gd = owner * n_pad + (np.arange(n) - bounds[owner])  # global padded slot

    # per-core edge streams (src owned by core), grouped by global dst tile
    per_core = []
    for c in range(NC):
        m = (src >= bounds[c]) & (src < bounds[c + 1])
        es = (src[m] - bounds[c]).astype(np.int64)   # local z row
        gde = gd[dst[m]]
        ec = coef[m]
        order = np.argsort(gde, kind="stable")
        es, ec, gde = es[order], ec[order], gde[order]
        tb = np.searchsorted(gde, np.arange(0, NC * n_pad + 1, 128))
        per_core.append((es, ec, gde, tb))

    # shared chunk schedule: CH[g] = max over cores of chunks for tile g
    CH = []
    for g in range(TF):
        mx = 1
        for c in range(NC):
            tb = per_core[c][3]
            cnt = int(tb[g + 1] - tb[g])
            mx = max(mx, (cnt + 127) // 128)
        CH.append(mx)
    TOT = sum(CH)

    # per-graph 1/count for mean pooling
    gcnt = np.bincount(batch, minlength=G).astype(np.float32)
    ginv = 1.0 / np.maximum(gcnt, 1.0)

    in_maps = []
    for c in range(NC):
        es, ec, gde, tb = per_core[c]
        gsrc = np.zeros(TOT * 128, dtype=np.int16)
        dlv = np.zeros(TOT * 128, dtype=np.float32)
        cfv = np.zeros(TOT * 128, dtype=np.float32)
        off = 0
        for g in range(TF):
            a, b = int(tb[g]), int(tb[g + 1])
            cnt = b - a
            gsrc[off:off + cnt] = es[a:b]
            dlv[off:off + cnt] = (gde[a:b] - g * 128).astype(np.float32)
            cfv[off:off + cnt] = ec[a:b]
            off += CH[g] * 128
        gidx_sb = _wrap16(gsrc, TOT * 8).astype(np.int16)
        dlcf = np.zeros((128, 2 * TOT), dtype=np.float32)
        dlcf[:, :TOT] = dlv.reshape(TOT, 128).T
        dlcf[:, TOT:] = cfv.reshape(TOT, 128).T

        lo, hi = int(bounds[c]), int(bounds[c + 1])
        xT = np.zeros((D_IN, n_pad), dtype=BF)
        xT[:, : hi - lo] = x[lo:hi].T.astype(BF)

        sc = np.zeros((128, T), dtype=np.float32)
        rows = np.arange(hi - lo)
        sc[rows % 128, rows // 128] = selfc[lo:hi]

        pp = np.zeros((128, T * 16), dtype=np.float32)
        gl = batch[lo:hi] - c * GP            # local graph id per node
        giv = ginv[c * GP:(c + 1) * GP]
        pp[rows % 128, (rows // 128) * 16 + gl] = giv[gl]

        bias = np.zeros((1, 2 * DH), dtype=BF)
        bias[0, :DH] = b1.astype(BF)
        bias[0, DH:] = b2.astype(BF)

        in_maps.append({
            "xT": xT,
            "w1": W1.astype(BF),
            "w2": W2.astype(BF),
            "pp": pp.astype(BF),
            "sc": sc,
            "bias": bias,
            "wfc": np.ascontiguousarray(np.broadcast_to(w_fc[:, 0][None, :], (16, DH)).astype(np.float32)),
            "bfc": np.full((16, 1), float(b_fc[0]), dtype=np.float32),
            "gidx": gidx_sb,
            "dlcf": dlcf,
        })

    return in_maps, n_pad, tuple(CH)


# ---------------------------------------------------------------- device build

_CACHE = {}


def build(n_pad, CH):
    key = (n_pad, CH)
    if key in _CACHE:
        return _CACHE[key]
    T = n_pad // 128
    TF = NC * T
    TOT = sum(CH)
    GRP = 8  # chunks per dma_gather (1024 idxs; hard per-call limit)

    nc = bacc.Bacc(dynamic_dma_scratch_size=98304)
    xT_in = nc.dram_tensor("xT", [D_IN, n_pad], BF16, kind="ExternalInput")
    w1_in = nc.dram_tensor("w1", [D_IN, DH], BF16, kind="ExternalInput")
    w2_in = nc.dram_tensor("w2", [DH, DH], BF16, kind="ExternalInput")
    pp_in = nc.dram_tensor("pp", [128, T * 16], BF16, kind="ExternalInput")
    sc_in = nc.dram_tensor("sc", [128, T], FP32, kind="ExternalInput")
    bias_in = nc.dram_tensor("bias", [1, 2 * DH], BF16, kind="ExternalInput")
    wfc_in = nc.dram_tensor("wfc", [16, DH], FP32, kind="ExternalInput")
    bfc_in = nc.dram_tensor("bfc", [16, 1], FP32, kind="ExternalInput")
    gidx_in = nc.dram_tensor("gidx", [128, TOT * 8], I16, kind="ExternalInput")
    dlcf_in = nc.dram_tensor("dlcf", [128, 2 * TOT], FP32, kind="ExternalInput")
    out = nc.dram_tensor("out", [16, 1], FP32, kind="ExternalOutput")

    zloc = [nc.dram_tensor(f"z{l}loc", [n_pad, DH], BF16) for l in (1, 2)]
    part = [nc.dram_tensor(f"p{l}art", [NC * n_pad, DH], BF16) for l in (1, 2)]
    agg = [nc.dram_tensor(f"a{l}gg", [n_pad, DH], BF16) for l in (1, 2)]
    zv = [t.rearrange("(t p) f -> p t f", p=128) for t in zloc]
    pv = [t.rearrange("(g p) f -> p g f", p=128) for t in part]
    av = [t.rearrange("(t p) f -> p t f", p=128) for t in agg]

    with TileContext(nc) as tc:
        with (
            tc.tile_pool(name="const", bufs=1) as const,
            tc.tile_pool(name="gp", bufs=3) as gp,
            tc.tile_pool(name="sp", bufs=6) as sp,
            tc.tile_pool(name="pw", bufs=2) as pw,
            tc.tile_pool(name="ap", bufs=2) as app,
            tc.tile_pool(name="hp", bufs=3) as hp,
            tc.tile_pool(name="tp", bufs=4) as tp,
            tc.tile_pool(name="fp", bufs=1) as fp,
            tc.tile_pool(name="psA", bufs=2, space="PSUM") as psA,
            tc.tile_pool(name="psM", bufs=3, space="PSUM") as psM,
            tc.tile_pool(name="psT", bufs=2, space="PSUM") as psT,
            tc.tile_pool(name="psP", bufs=1, space="PSUM") as psP,
        ):
            # ---- constant loads
            xT_sb = const.tile([128, 4, n_pad], BF16)
            for k in range(4):
                nc.sync.dma_start(out=xT_sb[:, k, :], in_=xT_in[k * 128:(k + 1) * 128, :])
            w1_sb = const.tile([128, 4, DH], BF16)
            for k in range(4):
                nc.sync.dma_start(out=w1_sb[:, k, :], in_=w1_in[k * 128:(k + 1) * 128, :])
            w2_sb = const.tile([128, 2, DH], BF16)
            for k in range(2):
                nc.sync.dma_start(out=w2_sb[:, k, :], in_=w2_in[k * 128:(k + 1) * 128, :])
            pp_sb = const.tile([128, T * 16], BF16)
            nc.sync.dma_start(out=pp_sb[:], in_=pp_in[:])
            sc_sb = const.tile([128, T], FP32)
            nc.sync.dma_start(out=sc_sb[:], in_=sc_in[:])
            bias_sb = const.tile([1, 2 * DH], BF16)
            nc.sync.dma_start(out=bias_sb[:], in_=bias_in[:])
            wfc_sb = const.tile([16, DH], FP32)
            nc.sync.dma_start(out=wfc_sb[:], in_=wfc_in[:])
            bfc_sb = const.tile([16, 1], FP32)
            nc.sync.dma_start(out=bfc_sb[:], in_=bfc_in[:])
            gidx_sb = const.tile([128, TOT * 8], I16)
            nc.sync.dma_start(out=gidx_sb[:], in_=gidx_in[:])
            dlcf_sb = const.tile([128, 2 * TOT], FP32)
            nc.sync.dma_start(out=dlcf_sb[:], in_=dlcf_in[:])

            ident = const.tile([128, 128], BF16)
            make_identity(nc, ident[:])
            ones_sb = const.tile([1, 128], BF16)
            nc.vector.memset(ones_sb[:], 1.0)
            iota_i = const.tile([128, 128], I32)
            nc.gpsimd.iota(iota_i[:], pattern=[[1, 128]], base=0, channel_multiplier=0)
            iota_f = const.tile([128, 128], BF16)
            nc.vector.tensor_copy(iota_f[:], iota_i[:])

            # broadcast bias rows to all 128 partitions (outer product with ones)
            biasb = const.tile([128, 2, DH], BF16)
            for l in range(2):
                pb = psA.tile([128, DH], FP32, space="PSUM", tag="psA")
                nc.tensor.matmul(out=pb[:], lhsT=ones_sb[:],
                                 rhs=bias_sb[0:1, l * DH:(l + 1) * DH],
                                 start=True, stop=True)
                nc.vector.tensor_copy(biasb[:, l, :], pb[:])

            # persistent z (bf16) per layer, also the staging for DRAM writes
            z1sb = const.tile([128, T, DH], BF16)
            z2sb = const.tile([128, T, DH], BF16)
            zsb = [z1sb, z2sb]

            # ---- phase A: z1 = x @ W1 (own nodes)
            for t in range(T):
                acc = psA.tile([128, DH], FP32, space="PSUM", tag="psA")
                for k in range(4):
                    nc.tensor.matmul(
                        out=acc[:], lhsT=xT_sb[:, k, t * 128:(t + 1) * 128],
                        rhs=w1_sb[:, k, :], start=(k == 0), stop=(k == 3))
                nc.vector.tensor_copy(zsb[0][:, t, :], acc[:])
            for w in range(0, T, 8):
                e = min(T, w + 8)
                nc.sync.dma_start(out=zv[0][:, w:e, :], in_=zsb[0][:, w:e, :])

            # ---- two GCN layers: scatter -> ReduceScatter -> self+bias+relu
            for l in range(2):
                gtile = [None, None]  # (group id, tile)

                def get_msg(q, l=l, gtile=gtile):
                    grp = q // GRP
                    if gtile[0] != grp:
                        sz = min(GRP, TOT - grp * GRP)
                        gt = gp.tile([128, sz, DH], BF16, tag="g")
                        nc.gpsimd.dma_gather(
                            out_ap=gt[:],
                            in_ap=zloc[l][:],
                            idxs_ap=gidx_sb[:, grp * GRP * 8:(grp * GRP + sz) * 8],
                            num_idxs=sz * 128,
                            num_idxs_reg=sz * 128,
                            elem_size=DH,
                        )
                        gtile[0], gtile[1] = grp, gt
                    return gtile[1][:, q % GRP, :]

                off = 0
                pstage = None
                for g in range(TF):
                    ch = CH[g]
                    acc = psM.tile([128, DH], FP32, space="PSUM", tag="psM")
                    for j in range(ch):
                        q = off + j
                        msg = get_msg(q)
                        S = sp.tile([128, 128], BF16, tag="S")
                        nc.vector.tensor_scalar(
                            out=S[:], in0=iota_f[:],
                            scalar1=dlcf_sb[:, q:q + 1],
                            scalar2=dlcf_sb[:, TOT + q:TOT + q + 1],
                            op0=mybir.AluOpType.is_equal,
                            op1=mybir.AluOpType.mult)
                        nc.tensor.matmul(out=acc[:], lhsT=S[:], rhs=msg,
                                         start=(j == 0), stop=(j == ch - 1))
                    if g % 8 == 0:
                        pstage = pw.tile([128, 8, DH], BF16, tag="pst")
                    nc.scalar.activation(out=pstage[:, g % 8, :], in_=acc[:],
                                         func=mybir.ActivationFunctionType.Copy)
                    if g % 8 == 7:
                        nc.sync.dma_start(out=pv[l][:, g - 7:g + 1, :],
                                          in_=pstage[:])
                    off += ch

                nc.gpsimd.collective_compute(
                    "ReduceScatter", mybir.AluOpType.add,
                    ins=[part[l][:]], outs=[agg[l][:]],
                    replica_groups=[list(range(NC))])

                # post-RS: h = relu(agg + selfc*z + bias); layer0 also z2 = h@W2
                for w in range(0, T, 8):
                    e = min(T, w + 8)
                    ast = app.tile([128, 8, DH], BF16, tag="agg")
                    nc.sync.dma_start(out=ast[:, :e - w, :], in_=av[l][:, w:e, :])
                    for t in range(w, e):
                        tmp = tp.tile([128, DH], BF16, tag="tmp")
                        nc.vector.tensor_scalar_mul(
                            tmp[:], zsb[l][:, t, :], sc_sb[:, t:t + 1])
                        sm = tp.tile([128, DH], BF16, tag="sm")
                        nc.vector.tensor_tensor(out=sm[:], in0=tmp[:],
                                                in1=ast[:, t - w, :],
                                                op=mybir.AluOpType.add)
                        sm2 = tp.tile([128, DH], BF16, tag="sm2")
                        nc.vector.tensor_tensor(out=sm2[:], in0=sm[:],
                                                in1=biasb[:, l, :],
                                                op=mybir.AluOpType.add)
                        h = hp.tile([128, DH], BF16, tag="h")
                        nc.scalar.activation(out=h[:], in_=sm2[:],
                                             func=mybir.ActivationFunctionType.Relu)
                        if l == 0:
                            hT = []
                            for half in range(2):
                                pt = psT.tile([128, 128], BF16, space="PSUM", tag="psT")
                                nc.tensor.transpose(
                                    out=pt[:], in_=h[:, half * 128:(half + 1) * 128],
                                    identity=ident[:])
                                ht = tp.tile([128, 128], BF16, tag="hT")
                                nc.vector.tensor_copy(ht[:], pt[:])
                                hT.append(ht)
                            accz = psA.tile([128, DH], FP32, space="PSUM", tag="psA")
                            for half in range(2):
                                nc.tensor.matmul(out=accz[:], lhsT=hT[half][:],
                                                 rhs=w2_sb[:, half, :],
                                                 start=(half == 0), stop=(half == 1))
                            nc.vector.tensor_copy(zsb[1][:, t, :], accz[:])
                        else:
                            if t == 0:
                                pool_acc = psP.tile([16, DH], FP32, space="PSUM", tag="psP")
                            nc.tensor.matmul(out=pool_acc[:],
                                             lhsT=pp_sb[:, t * 16:(t + 1) * 16],
                                             rhs=h[:], start=(t == 0), stop=(t == T - 1),
                                             skip_group_check=True)
                    if l == 0:
                        nc.sync.dma_start(out=zv[1][:, w:e, :], in_=zsb[1][:, w:e, :])

            # ---- fc head: out = pooled @ w_fc + b_fc
            pooled = fp.tile([16, DH], FP32)
            nc.vector.tensor_copy(pooled[:], pool_acc[:])
            prod = fp.tile([16, DH], FP32)
            nc.vector.tensor_tensor(out=prod[:], in0=pooled[:], in1=wfc_sb[:],
                                    op=mybir.AluOpType.mult)
            red = fp.tile([16, 1], FP32)
            nc.vector.reduce_sum(red[:], prod[:], axis=mybir.AxisListType.X)
            outv = fp.tile([16, 1], FP32)
            nc.vector.tensor_scalar_add(outv[:], red[:], bfc_sb[:])
            nc.sync.dma_start(out=out[:], in_=outv[:])

    nc.finalize()
    _CACHE[key] = nc
    return nc


# ---------------------------------------------------------------- entry points

def _run(inputs, trace=False):
    in_maps, n_pad, CH = prep(**inputs)
    nc = build(n_pad, CH)
    r = run_bass_kernel_spmd(nc, in_maps, list(range(NC)), trace=trace)
    parts = [r.results[c]["out"][:, 0] for c in range(NC)]
    return np.concatenate(parts).astype(np.float32), r


def kernel(**inputs):
    out, _ = _run(inputs, trace=False)
    return out


def kernel_traced(**inputs):
    out, r = _run(inputs, trace=True)
    return out, r
